# revision 25
# baseline (speedup 1.0000x reference)
"""Trainium2 Bass kernel for nn_CellLineGraphEncoder (GENConv x3 + pooling).

Strategy (8 NeuronCores, SPMD):
  - Nodes are sharded: core k owns 49 blocks of 128 nodes (6272 nodes).
  - Edges are sharded by dst node owner; per (core, dst-block) the edges are
    split into lo/hi halves by gather-table row parity (int16 index limit)
    and each half is padded to a uniform chunk count so a single program
    fits all cores.
  - Per layer: all-gather a bf16 z-table (conv input features + b_edge,
    pair-packed to 256B rows), bulk-gather z[src] rows per edge via
    dma_gather, compute softmax-aggregation messages on ACT/DVE, segment-
    reduce per dst-block with one-hot matmuls on PE (S1 = sum exp,
    S2 = sum exp*msg), then agg = S2/(S1+eps), residual, and the
    64->128->64 MLP with feature-dim LayerNorm.
  - Performance notes vs the naive version:
    * one-hot scatter matrices are built in ONE batched is_equal per block
      (not per chunk) to cut DVE time (DVE tensor_tensor ops lock the
      SBUF port pair that SWDGE descriptor generation needs, so DVE load
      directly inflates dma_gather time on GpSimd).
    * av*W_edge is precomputed on the host per edge slot and streamed from
      DRAM via HWDGE (layer-invariant), removing a broadcast multiply.
    * the whole node MLP/LN pipeline runs in bf16 (weights, stats and
      broadcast matmuls), avoiding fp32 LOW_HIGH double-pass matmuls.
    * the ACT function table is pinned to natural_log_exp_and_others so
      the engine never reloads tables when Ln/Exp/Relu/Square interleave.
    * zero biases / unit gains (per the problem's fills) skip their ops at
      build time; general values still supported via the slow path.
  - Softmax max-subtraction is dropped (messages are bounded ~7, exp is safe;
    softmax is shift-invariant so the result matches the reference).
  - Final avg/max pooling is reduced per core on device; the cross-core
    combine and the last [1,64] @ [64,64] linear run on host.
"""

import math
import numpy as np
import ml_dtypes

import concourse.bass as bass
import concourse.bacc as bacc
import concourse.mybir as mybir
import concourse.tile as tile
from concourse.bass_utils import run_bass_kernel_spmd
from concourse.masks import make_identity

P = 128
NCORES = 8
N_NODES = 50000
N_EDGES = 1000000
HID = 64
L = 3
NBLK = 49                   # dst-node blocks per core
NPC = NBLK * P              # nodes per core (6272)
NTOT = NCORES * NPC         # padded node count (50176)
EPS_MSG = 1e-7
EPS_SM = 1e-16
LN_EPS = 1e-5

F32 = mybir.dt.float32
BF16 = mybir.dt.bfloat16
I16 = mybir.dt.int16
I32 = mybir.dt.int32
OP = mybir.AluOpType
ACT = mybir.ActivationFunctionType

# 6272 = 12*512 + 128
SLICES = [(i * 512, 512) for i in range(12)] + [(6144, 128)]


def _table_row(node):
    """Row of a node in the all-gathered z table (p-major per-core layout)."""
    kc = node // NPC
    loc = node - kc * NPC
    p = loc % P
    b = loc // P
    return kc * NPC + p * NBLK + b


def _prep_edges(edge_index, edge_attr):
    """Pair-packed table: 2 nodes per 256B row -> 25088 rows, unsigned int16
    idx covers the whole table. Per (core, block) edges are split by src-row
    parity (even rows first, then odd) so each chunk reads the low or high
    64 features of its gathered pair row via a pure AP offset."""
    src = edge_index[0].astype(np.int64)
    dst = edge_index[1].astype(np.int64)
    core = dst // NPC
    blk = (dst % NPC) // P
    dl = ((dst % NPC) % P).astype(np.float32)
    row = _table_row(src)
    pr = row >> 1               # pair row, 0..25087
    par = (row & 1).astype(np.int64)
    g = core * NBLK + blk
    av = edge_attr[:, 0].astype(np.float32)

    gh = g * 2 + par
    counts = np.bincount(gh, minlength=NCORES * NBLK * 2)
    cnt2 = counts.reshape(NCORES, NBLK, 2)
    cnt_e = cnt2[:, :, 0].max(axis=0).astype(int)
    cnt_o = cnt2[:, :, 1].max(axis=0).astype(int)
    K_E = [max(1, int(math.ceil(c / P))) for c in cnt_e]
    K_O = [max(1, int(math.ceil(c / P))) for c in cnt_o]
    choff = np.zeros(NBLK + 1, np.int64)
    for b in range(NBLK):
        choff[b + 1] = choff[b] + K_E[b] + K_O[b]
    nchk = int(choff[NBLK])
    # valid gather span per block (trailing strip hits only -1 slots)
    reg = [K_E[b] * P + int(cnt_o[b]) for b in range(NBLK)]

    order = np.argsort(gh, kind="stable")
    starts = np.zeros(NCORES * NBLK * 2 + 1, np.int64)
    np.cumsum(counts, out=starts[1:])
    pr_s, dl_s, av_s = pr[order], dl[order], av[order]

    nslot = nchk * P
    idx_c, dl_c, av_c = [], [], []
    for c in range(NCORES):
        idxa = np.full(nslot, -1, np.int32)
        dla = np.full(nslot, -1.0, np.float32)
        ava = np.zeros(nslot, np.float32)
        for b in range(NBLK):
            for half, K_half, cmax_a in ((0, K_E, cnt_e), (1, K_O, cnt_o)):
                gi = (c * NBLK + b) * 2 + half
                s, e = starts[gi], starts[gi + 1]
                n = e - s
                o = (choff[b] + (K_E[b] if half else 0)) * P
                cmax = cmax_a[b]
                idxa[o:o + cmax] = 0
                idxa[o:o + n] = pr_s[s:e]
                dla[o:o + n] = dl_s[s:e]
                ava[o:o + n] = av_s[s:e]
        # value arrays: slot j -> (lane j%128, chunk j//128)
        dl_c.append(np.ascontiguousarray(dla.reshape(nchk, P).T))
        av_c.append(np.ascontiguousarray(ava.reshape(nchk, P).T))
        # index array: slot j -> (partition j%16, free j//16), replicated x8
        i16 = idxa.astype(np.int16).reshape(-1, 16).T   # [16, nslot/16]
        idx_c.append(np.ascontiguousarray(np.tile(i16, (8, 1))))
    return K_E, K_O, reg, idx_c, dl_c, av_c


def _build_flags(data):
    return {
        "b_node": bool(np.any(data["b_node"])),
        "b_edge": bool(np.any(data["b_edge"])),
        "b1": bool(np.any(data["b1"])),
        "b2": bool(np.any(data["b2"])),
        "g1": bool(np.any(data["ln_g"] != 1.0) or np.any(data["ln_b"])),
        "gn": bool(np.any(data["norm_g"] != 1.0) or np.any(data["norm_b"])),
        "tval": float(data["t"][0]) if np.all(data["t"] == data["t"][0])
                else None,
    }


def _pin_act_tables():
    """Make every ACT function resolve to natural_log_exp_and_others so the
    engine loads one table and never swaps (Ln/Exp/Relu/Square interleave
    heavily here). Only the set-choice hint is changed; ids stay aligned
    with act_info.json. Returns an undo callable."""
    orig = bacc.get_activation_tables
    PIN = "natural_log_exp_and_others"

    def patched(arch):
        tabs = orig(arch)
        if PIN not in tabs:
            return tabs
        keep = tabs[PIN]
        return {n: (f if n == PIN else (f - keep)) for n, f in tabs.items()}

    bacc.get_activation_tables = patched
    return lambda: setattr(bacc, "get_activation_tables", orig)


def _build(K_E, K_O, reg, flags):
    choff = [0]
    for b in range(NBLK):
        choff.append(choff[b] + K_E[b] + K_O[b])
    nchk = choff[NBLK]
    nslot = nchk * P
    KMAX = max(K_E[b] + K_O[b] for b in range(NBLK))
    HAS_BN = flags["b_node"]
    HAS_BE = flags["b_edge"]
    HAS_B1 = flags["b1"]
    HAS_B2 = flags["b2"]
    HAS_G1 = flags["g1"]
    HAS_GN = flags["gn"]
    TVAL = flags["tval"]

    unpin = _pin_act_tables()
    try:
        nc = bacc.Bacc("TRN2", target_bir_lowering=False, debug=False,
                       num_devices=NCORES, num_swdge_queues=4,
                       dynamic_dma_scratch_size=32768)
        d_idx = nc.dram_tensor("eidx16", [P, nslot // 16], I16,
                               kind="ExternalInput").ap()
        d_oh = nc.dram_tensor("ohm", [P, nchk, P], BF16,
                              kind="ExternalInput").ap()
        d_eaw = nc.dram_tensor("eaw", [P, nchk, HID], BF16,
                               kind="ExternalInput").ap()
        d_x16 = nc.dram_tensor("x16", [16, NPC], BF16,
                               kind="ExternalInput").ap()
        d_wnode = nc.dram_tensor("wnode", [16, HID], BF16,
                                 kind="ExternalInput").ap()
        d_w1 = nc.dram_tensor("w1", [L, HID, 2 * HID], BF16,
                              kind="ExternalInput").ap()
        d_w2 = nc.dram_tensor("w2", [L, 2 * HID, HID], BF16,
                              kind="ExternalInput").ap()
        if HAS_BN:
            d_bnode = nc.dram_tensor("bnode", [HID, 1], F32,
                                     kind="ExternalInput").ap()
        if HAS_BE:
            d_benm = nc.dram_tensor("benm", [P, HID], F32,
                                    kind="ExternalInput").ap()
        if HAS_B1:
            d_b1c = nc.dram_tensor("b1c", [2 * HID, L], F32,
                                   kind="ExternalInput").ap()
        if HAS_B2:
            d_b2c = nc.dram_tensor("b2c", [HID, L], F32,
                                   kind="ExternalInput").ap()
        if HAS_G1:
            d_g1c = nc.dram_tensor("g1c", [2 * HID, L], F32,
                                   kind="ExternalInput").ap()
            d_be1c = nc.dram_tensor("be1c", [2 * HID, L], F32,
                                    kind="ExternalInput").ap()
        if HAS_GN:
            d_ngc = nc.dram_tensor("ngc", [HID, L], F32,
                                   kind="ExternalInput").ap()
            d_nbc = nc.dram_tensor("nbc", [HID, L], F32,
                                   kind="ExternalInput").ap()
        if TVAL is None:
            d_tc = nc.dram_tensor("tcol", [P, L], F32,
                                  kind="ExternalInput").ap()
        d_out = nc.dram_tensor("pool_out", [HID, 2], F32,
                               kind="ExternalOutput").ap()

        with tile.TileContext(nc) as tc:
            with (
                tc.tile_pool(name="cpool", bufs=1) as cpool,
                tc.tile_pool(name="epool", bufs=3) as epool,
                tc.tile_pool(name="vpool", bufs=3) as vpool,
                tc.tile_pool(name="ohpool", bufs=3) as ohpool,
                tc.tile_pool(name="wpool", bufs=3) as wpool,
                tc.tile_pool(name="npool", bufs=3) as npool,
                tc.tile_pool(name="mpool", bufs=2) as mpool,
                tc.tile_pool(name="spool", bufs=2) as spool,
                tc.tile_pool(name="pmm", bufs=2, space="PSUM") as pmm,
                tc.tile_pool(name="pmlp", bufs=2, space="PSUM") as pmlp,
                tc.tile_pool(name="ptp", bufs=1, space="PSUM") as ptp,
                tc.tile_pool(name="paux", bufs=3, space="PSUM") as paux,
                tc.tile_pool(name="dpool", bufs=1, space="DRAM") as dpool,
            ):
                # ---- constants / persistent state ----
                ident = cpool.tile([P, P], F32)
                make_identity(nc, ident[:])
                identb = cpool.tile([P, P], BF16)
                nc.vector.tensor_copy(identb[:], ident[:])
                ones_row_bf = cpool.tile([1, P], BF16)
                nc.vector.memset(ones_row_bf[:], 1.0)
                inv64b = cpool.tile([P, 1], BF16)
                nc.vector.memset(inv64b[:], 1.0 / 64)
                inv128b = cpool.tile([P, 1], BF16)
                nc.vector.memset(inv128b[:], 1.0 / 128)
                epsm_col = cpool.tile([P, 1], F32)
                nc.vector.memset(epsm_col[:], EPS_MSG)
                lneps_col = cpool.tile([P, 1], F32)
                nc.vector.memset(lneps_col[:], LN_EPS)

                def load(shape, dtype, src_ap, name):
                    t = cpool.tile(shape, dtype, name=name)
                    nc.sync.dma_start(out=t[:], in_=src_ap)
                    return t

                idx16 = load([P, nslot // 16], I16, d_idx, "idx16_s")
                wnodeb = load([16, HID], BF16, d_wnode, "wnode_s")
                w1b = [load([HID, 2 * HID], BF16, d_w1[l], f"w1_{l}")
                       for l in range(L)]
                w2b = [load([2 * HID, HID], BF16, d_w2[l], f"w2_{l}")
                       for l in range(L)]
                bnode = load([HID, 1], F32, d_bnode, "bnode_s") if HAS_BN \
                    else None
                benm = load([P, HID], F32, d_benm, "benm_s") if HAS_BE \
                    else None
                b1c = load([2 * HID, L], F32, d_b1c, "b1c_s") if HAS_B1 \
                    else None
                b2c = load([HID, L], F32, d_b2c, "b2c_s") if HAS_B2 else None
                if HAS_G1:
                    g1c = load([2 * HID, L], F32, d_g1c, "g1c_s")
                    be1c = load([2 * HID, L], F32, d_be1c, "be1c_s")
                if HAS_GN:
                    ngc = load([HID, L], F32, d_ngc, "ngc_s")
                    nbc = load([HID, L], F32, d_nbc, "nbc_s")
                tcol = load([P, L], F32, d_tc, "tcol_s") if TVAL is None \
                    else None

                h = cpool.tile([HID, NPC], BF16)        # feature-major state
                zf = cpool.tile([HID, NPC], BF16)       # relu(LN(h)) scratch
                in1T_bf = cpool.tile([HID, NPC], BF16)  # MLP input, fmajor
                zrows = cpool.tile([P, NBLK * HID], F32)  # z node-major rows
                zbb = cpool.tile([P, NBLK, HID], BF16)  # z + b_edge (packed)
                nc.vector.memset(zbb[:], 0.0)
                gts = [cpool.tile([P, KMAX, 2 * HID], BF16, name=f"gt{i}")
                       for i in range(4)]
                for g_ in gts:
                    nc.vector.memset(g_[:], 0.0)

                zbounce = [dpool.tile([NPC, HID], BF16, name=f"zbounce{l}")
                           for l in range(L)]
                ztable = [dpool.tile([NTOT, HID], BF16, name=f"ztable{l}",
                                     addr_space="Shared") for l in range(L)]

                # ---- h0 = x @ W_node (+ b_node), feature-major bf16 ----
                x16b, x16_free = tc.tile([16, NPC], BF16, name="x16b")
                nc.sync.dma_start(out=x16b[:], in_=d_x16)
                for c0, w in SLICES:
                    ph0 = pmlp.tile([HID, 512], F32, name="p_h0", tag="mlp")
                    nc.tensor.matmul(ph0[:, :w], lhsT=wnodeb[:],
                                     rhs=x16b[:, c0:c0 + w],
                                     start=True, stop=True)
                    if HAS_BN:
                        nc.vector.tensor_scalar(
                            out=h[:, c0:c0 + w], in0=ph0[:, :w],
                            scalar1=bnode[:, 0:1], scalar2=None, op0=OP.add)
                    else:
                        nc.vector.tensor_copy(h[:, c0:c0 + w], ph0[:, :w])
                x16_free()

                # ---- feature-major LayerNorm + relu (stats over P dim) ----
                def ln_apply(dst, src, nfeat, g_col, b_col, c0, w, pfx):
                    invc = inv64b if nfeat == HID else inv128b
                    sq = spool.tile([nfeat, 512], BF16, name=pfx + "sq",
                                    tag=pfx + "sq")
                    nc.scalar.activation(sq[:, :w], src[:nfeat, c0:c0 + w],
                                         ACT.Square)
                    pm = paux.tile([1, 512], F32, name=pfx + "pm", tag="aux")
                    pq = paux.tile([1, 512], F32, name=pfx + "pq", tag="aux")
                    nc.tensor.matmul(pm[:, :w], lhsT=invc[:nfeat, :],
                                     rhs=src[:nfeat, c0:c0 + w],
                                     start=True, stop=True)   # mean
                    nc.tensor.matmul(pq[:, :w], lhsT=invc[:nfeat, :],
                                     rhs=sq[:, :w], start=True, stop=True)
                    tmp = spool.tile([1, 512], F32, name=pfx + "tmp",
                                     tag="sttmp")
                    nc.scalar.activation(tmp[:, :w], pm[:, :w], ACT.Square)
                    stA = spool.tile([1, 512], F32, name=pfx + "stA",
                                     tag="stA")
                    nc.vector.tensor_tensor(out=stA[:, :w], in0=pq[:, :w],
                                            in1=tmp[:, :w], op=OP.subtract)
                    # rstd = exp(-0.5*ln(var+eps)); both funcs in one table
                    nc.scalar.activation(stA[:, :w], stA[:, :w], ACT.Ln,
                                         bias=lneps_col[:1, :])
                    stAb = spool.tile([1, 512], BF16, name=pfx + "stAb",
                                      tag="stAb")
                    nc.scalar.activation(stAb[:, :w], stA[:, :w], ACT.Exp,
                                         scale=-0.5)
                    stBb = spool.tile([1, 512], BF16, name=pfx + "stBb",
                                      tag="stBb")
                    nc.vector.tensor_tensor(out=stBb[:, :w], in0=pm[:, :w],
                                            in1=stAb[:, :w], op=OP.mult)
                    pA = paux.tile([nfeat, 512], F32, name=pfx + "pA",
                                   tag="aux")
                    pB = paux.tile([nfeat, 512], F32, name=pfx + "pB",
                                   tag="aux")
                    nc.tensor.matmul(pA[:, :w], lhsT=ones_row_bf[:, :nfeat],
                                     rhs=stAb[:, :w], start=True, stop=True)
                    nc.tensor.matmul(pB[:, :w], lhsT=ones_row_bf[:, :nfeat],
                                     rhs=stBb[:, :w], start=True, stop=True)
                    u = spool.tile([nfeat, 512], BF16, name=pfx + "u",
                                   tag=pfx + "u")
                    nc.vector.tensor_tensor(out=u[:, :w],
                                            in0=src[:nfeat, c0:c0 + w],
                                            in1=pA[:, :w], op=OP.mult)
                    nc.vector.tensor_tensor(out=u[:, :w], in0=u[:, :w],
                                            in1=pB[:, :w], op=OP.subtract)
                    if g_col is None:
                        nc.scalar.activation(dst[:nfeat, c0:c0 + w],
                                             u[:, :w], ACT.Relu)
                    else:
                        nc.scalar.activation(dst[:nfeat, c0:c0 + w],
                                             u[:, :w], ACT.Relu,
                                             scale=g_col, bias=b_col)

                # ---- z-prep: transpose z to node-major rows + table pack
                def zprep_blocks(lay_z, b0, b1):
                    zsrc = h if lay_z == 0 else zf
                    for b in range(b0, b1):
                        ptz = ptp.tile([P, HID], BF16, name="ptz", tag="tp")
                        nc.tensor.transpose(ptz[:],
                                            zsrc[:, b * P:(b + 1) * P],
                                            identb[:HID, :HID])
                        nc.vector.tensor_copy(zrows[:, b * HID:(b + 1) * HID],
                                              ptz[:])
                    nb = b1 - b0
                    if HAS_BE:
                        nc.vector.tensor_tensor(
                            out=zbb[:, b0:b1, :],
                            in0=zrows[:, b0 * HID:b1 * HID]
                                .rearrange("p (b f) -> p b f", b=nb),
                            in1=benm[:].rearrange("p (o f) -> p o f", o=1)
                                .to_broadcast([P, nb, HID]),
                            op=OP.add)
                    else:
                        nc.scalar.activation(
                            zbb[:, b0:b1, :],
                            zrows[:, b0 * HID:b1 * HID]
                                .rearrange("p (b f) -> p b f", b=nb),
                            ACT.Copy)

                def table_publish(lay_z):
                    nc.sync.dma_start(
                        out=zbounce[lay_z][:]
                            .rearrange("(p b) f -> p (b f)", p=P),
                        in_=zbb[:])
                    nc.gpsimd.collective_compute(
                        "AllGather", OP.bypass,
                        replica_groups=[list(range(NCORES))],
                        ins=[zbounce[lay_z][:].opt()],
                        outs=[ztable[lay_z][:].opt()])

                # ---- layers ----
                for lay in range(L):
                    if lay == 0:
                        zprep_blocks(0, 0, NBLK)
                        table_publish(0)
                    tsc = (float(TVAL) if TVAL is not None
                           else tcol[:, lay:lay + 1])

                    # node MLP for a 512-node slice (interleaved into edges)
                    def mlp_slice(c0, w):
                        p1 = pmlp.tile([P, 512], F32, name="p1", tag="mlp")
                        nc.tensor.matmul(p1[:, :w], lhsT=w1b[lay][:],
                                         rhs=in1T_bf[:, c0:c0 + w],
                                         start=True, stop=True)
                        t1 = mpool.tile([P, 512], BF16, name="t1", tag="t1")
                        if HAS_B1:
                            nc.scalar.activation(t1[:, :w], p1[:, :w],
                                                 ACT.Identity,
                                                 bias=b1c[:, lay:lay + 1])
                        else:
                            nc.scalar.activation(t1[:, :w], p1[:, :w],
                                                 ACT.Copy)
                        r = mpool.tile([P, 512], BF16, name="mr", tag="mr")
                        ln_apply(r, t1, 2 * HID,
                                 g1c[:, lay:lay + 1] if HAS_G1 else None,
                                 be1c[:, lay:lay + 1] if HAS_G1 else None,
                                 0, w, "m")
                        p2 = pmlp.tile([HID, 512], F32, name="p2", tag="mlp")
                        nc.tensor.matmul(p2[:, :w], lhsT=w2b[lay][:],
                                         rhs=r[:, :w], start=True, stop=True)
                        if lay == 0:
                            if HAS_B2:
                                nc.vector.tensor_scalar(
                                    out=h[:, c0:c0 + w], in0=p2[:, :w],
                                    scalar1=b2c[:, 0:1], scalar2=None,
                                    op0=OP.add)
                            else:
                                nc.vector.tensor_copy(h[:, c0:c0 + w],
                                                      p2[:, :w])
                        else:
                            if HAS_B2:
                                conv = mpool.tile([HID, 512], BF16,
                                                  name="conv", tag="conv")
                                nc.vector.tensor_scalar(
                                    out=conv[:, :w], in0=p2[:, :w],
                                    scalar1=b2c[:, lay:lay + 1],
                                    scalar2=None, op0=OP.add)
                                nc.vector.tensor_tensor(
                                    out=h[:, c0:c0 + w],
                                    in0=h[:, c0:c0 + w],
                                    in1=conv[:, :w], op=OP.add)
                            else:
                                nc.vector.tensor_tensor(
                                    out=h[:, c0:c0 + w],
                                    in0=h[:, c0:c0 + w],
                                    in1=p2[:, :w], op=OP.add)

                    # ---- edge phase ----
                    for b in range(NBLK):
                        cbase = choff[b]
                        ke, ko = K_E[b], K_O[b]
                        Kb = ke + ko
                        gt = gts[b % 4]
                        nc.gpsimd.dma_gather(
                            out_ap=gt[:, 0:Kb, :],
                            in_ap=ztable[lay][:].rearrange(
                                "(m two) f -> m (two f)", two=2),
                            idxs_ap=idx16[:, cbase * 8:(cbase + Kb) * 8],
                            num_idxs=Kb * P,
                            num_idxs_reg=int(reg[b]),
                            elem_size=2 * HID, single_packet=False,
                            queue_num=b % 4)
                        eaw_t = wpool.tile([P, KMAX, HID], BF16, name="eawt",
                                           tag="eawt")
                        nc.sync.dma_start(
                            out=eaw_t[:, 0:Kb, :],
                            in_=d_eaw[:, cbase:cbase + Kb, :])
                        oh = ohpool.tile([P, KMAX, P], BF16, name="oh",
                                         tag="oh")
                        nc.sync.dma_start(
                            out=oh[:, 0:Kb, :],
                            in_=d_oh[:, cbase:cbase + Kb, :])
                        ea = epool.tile([P, KMAX, HID], BF16, name="ea",
                                        tag="ea")
                        val = vpool.tile([P, KMAX, 2, HID], BF16, name="val",
                                         tag="val")
                        for ofs, ncnk, fofs in ((0, ke, 0), (ke, ko, HID)):
                            nc.vector.tensor_tensor(
                                out=ea[:, ofs:ofs + ncnk, :],
                                in0=gt[:, ofs:ofs + ncnk, fofs:fofs + HID],
                                in1=eaw_t[:, ofs:ofs + ncnk, :], op=OP.add)
                        eas = ea[:, 0:Kb, :]
                        nc.scalar.activation(eas, eas, ACT.Relu,
                                             bias=epsm_col[:])   # msg
                        nc.scalar.activation(val[:, 0:Kb, 0, :], eas,
                                             ACT.Exp, scale=tsc)
                        nc.vector.tensor_tensor(
                            out=val[:, 0:Kb, 1, :],
                            in0=val[:, 0:Kb, 0, :],
                            in1=eas, op=OP.mult)
                        ps = pmm.tile([P, P], F32, name="ps", tag="mm")
                        for k in range(Kb):
                            nc.tensor.matmul(ps[:], lhsT=oh[:, k, :],
                                             rhs=val[:, k, :, :],
                                             start=(k == 0),
                                             stop=(k == Kb - 1))
                        rec = npool.tile([P, HID], F32, name="rec", tag="rec")
                        nc.vector.tensor_scalar(out=rec[:], in0=ps[:, 0:HID],
                                                scalar1=EPS_SM, scalar2=None,
                                                op0=OP.add)
                        nc.vector.reciprocal(rec[:], rec[:])
                        in1 = npool.tile([P, HID], F32, name="in1", tag="in1")
                        nc.vector.tensor_tensor(out=in1[:],
                                                in0=ps[:, HID:2 * HID],
                                                in1=rec[:], op=OP.mult)
                        nc.vector.tensor_tensor(
                            out=in1[:], in0=in1[:],
                            in1=zrows[:, b * HID:(b + 1) * HID], op=OP.add)
                        pti = ptp.tile([HID, P], F32, name="pti", tag="tp")
                        nc.tensor.transpose(pti[:], in1[:], ident[:])
                        nc.scalar.activation(
                            in1T_bf[:, b * P:(b + 1) * P], pti[:], ACT.Copy)
                        # run the MLP for a 512-node slice as soon as its 4
                        # blocks of in1T are ready (overlaps later gathers),
                        # then LN + transpose those blocks for the next
                        # layer's z-table so only DMA+AllGather remain at
                        # the layer boundary
                        if b % 4 == 3 or b == NBLK - 1:
                            j = b // 4 if b % 4 == 3 else 12
                            mlp_slice(*SLICES[j])
                            if lay + 1 < L:
                                ln_apply(zf, h, HID,
                                         ngc[:, lay + 1:lay + 2]
                                         if HAS_GN else None,
                                         nbc[:, lay + 1:lay + 2]
                                         if HAS_GN else None,
                                         *SLICES[j], "z")
                                zprep_blocks(lay + 1, 4 * j,
                                             min(4 * j + 4, NBLK))
                    if lay + 1 < L:
                        table_publish(lay + 1)

                # ---- final norm + pooling ----
                for c0, w in SLICES:
                    ln_apply(zf, h, HID,
                             ngc[:, 0:1] if HAS_GN else None,
                             nbc[:, 0:1] if HAS_GN else None, c0, w, "z")
                poolsb = cpool.tile([HID, 2], F32)
                nc.vector.tensor_reduce(out=poolsb[:, 0:1], in_=zf[:HID, :],
                                        axis=mybir.AxisListType.X, op=OP.add)
                nc.vector.tensor_reduce(out=poolsb[:, 1:2], in_=zf[:HID, :],
                                        axis=mybir.AxisListType.X, op=OP.max)
                nc.sync.dma_start(out=d_out, in_=poolsb[:])

        nc.finalize()
    finally:
        unpin()
    return nc


def make_in_maps(data, idx_c, dl_c, av_c):
    flags = _build_flags(data)
    x = data["x"].astype(np.float32)
    xpad = np.zeros((NTOT, 16), np.float32)
    xpad[:N_NODES] = x
    w_edge = data["W_edge"].astype(np.float32)[0]          # [64]
    bf = ml_dtypes.bfloat16
    in_maps = []
    for c in range(NCORES):
        xc = xpad[c * NPC:(c + 1) * NPC]
        # feature-major x: [16, NPC] with node order (p-major per block)
        xp = np.ascontiguousarray(xc.T.astype(bf))   # [16, NPC], node = b*P+p
        eaw = np.ascontiguousarray(
            (av_c[c][:, :, None] * w_edge[None, None, :]).astype(bf))
        lanes = np.arange(P, dtype=np.float32)
        ohm = np.ascontiguousarray(
            (dl_c[c][:, :, None] == lanes[None, None, :]).astype(bf))
        m = {
            "eidx16": idx_c[c],
            "ohm": ohm,
            "eaw": eaw,
            "x16": xp,
            "wnode": np.ascontiguousarray(data["W_node"].astype(bf)),
            "w1": np.ascontiguousarray(data["W1"].astype(bf)),
            "w2": np.ascontiguousarray(data["W2"].astype(bf)),
        }
        if flags["b_node"]:
            m["bnode"] = data["b_node"].astype(np.float32)[:, None].copy()
        if flags["b_edge"]:
            m["benm"] = np.tile(data["b_edge"].astype(np.float32)[None, :],
                                (P, 1))
        if flags["b1"]:
            m["b1c"] = np.ascontiguousarray(data["b1"].astype(np.float32).T)
        if flags["b2"]:
            m["b2c"] = np.ascontiguousarray(data["b2"].astype(np.float32).T)
        if flags["g1"]:
            m["g1c"] = np.ascontiguousarray(data["ln_g"].astype(np.float32).T)
            m["be1c"] = np.ascontiguousarray(
                data["ln_b"].astype(np.float32).T)
        if flags["gn"]:
            m["ngc"] = np.ascontiguousarray(
                data["norm_g"].astype(np.float32).T)
            m["nbc"] = np.ascontiguousarray(
                data["norm_b"].astype(np.float32).T)
        if flags["tval"] is None:
            m["tcol"] = np.tile(data["t"].astype(np.float32)[None, :],
                                (P, 1))
        in_maps.append(m)
    return in_maps


def kernel(x, edge_attr, edge_index, W_node, b_node, W_edge, b_edge, t,
           W1, b1, ln_g, ln_b, W2, b2, norm_g, norm_b, W_lin, b_lin):
    data = dict(x=np.asarray(x), edge_attr=np.asarray(edge_attr),
                W_node=np.asarray(W_node), b_node=np.asarray(b_node),
                W_edge=np.asarray(W_edge), b_edge=np.asarray(b_edge),
                t=np.asarray(t), W1=np.asarray(W1), b1=np.asarray(b1),
                ln_g=np.asarray(ln_g), ln_b=np.asarray(ln_b),
                W2=np.asarray(W2), b2=np.asarray(b2),
                norm_g=np.asarray(norm_g), norm_b=np.asarray(norm_b))
    K_E, K_O, reg, idx_c, dl_c, av_c = _prep_edges(
        np.asarray(edge_index), np.asarray(edge_attr))
    nc = _build(K_E, K_O, reg, _build_flags(data))
    in_maps = make_in_maps(data, idx_c, dl_c, av_c)
    res = run_bass_kernel_spmd(nc, in_maps, core_ids=list(range(NCORES)))
    outs = np.stack([np.asarray(r["pool_out"], np.float32)
                     for r in res.results])        # [8, 64, 2]
    sums = outs[:, :, 0].sum(axis=0)
    maxs = outs[:, :, 1].max(axis=0)
    avg = (sums / float(N_NODES)).reshape(32, 2).mean(axis=1)
    mx = maxs.reshape(32, 2).max(axis=1)
    emb = np.concatenate([avg, mx])[None, :].astype(np.float32)
    out = emb @ np.asarray(W_lin, np.float32) + np.asarray(b_lin, np.float32)
    return out.astype(np.float32)


# revision 32
# speedup vs baseline: 1.2738x; 1.2738x over previous
"""Trainium2 Bass kernel for nn_CellLineGraphEncoder (GENConv x3 + pooling).

Strategy (8 NeuronCores, SPMD):
  - Nodes are sharded: core k owns 49 blocks of 128 nodes (6272 nodes).
  - Edges are sharded by dst node owner; per (core, dst-block) the edges are
    split into lo/hi halves by gather-table row parity (int16 index limit)
    and each half is padded to a uniform chunk count so a single program
    fits all cores.
  - Per layer: all-gather a bf16 z-table (conv input features + b_edge,
    pair-packed to 256B rows), bulk-gather z[src] rows per edge via
    dma_gather, compute softmax-aggregation messages on ACT/DVE, segment-
    reduce per dst-block with one-hot matmuls on PE (S1 = sum exp,
    S2 = sum exp*msg), then agg = S2/(S1+eps), residual, and the
    64->128->64 MLP with feature-dim LayerNorm.
  - Performance notes vs the naive version:
    * one-hot scatter matrices are built in ONE batched is_equal per block
      (not per chunk) to cut DVE time (DVE tensor_tensor ops lock the
      SBUF port pair that SWDGE descriptor generation needs, so DVE load
      directly inflates dma_gather time on GpSimd).
    * av*W_edge is precomputed on the host per edge slot and streamed from
      DRAM via HWDGE (layer-invariant), removing a broadcast multiply.
    * the whole node MLP/LN pipeline runs in bf16 (weights, stats and
      broadcast matmuls), avoiding fp32 LOW_HIGH double-pass matmuls.
    * the ACT function table is pinned to natural_log_exp_and_others so
      the engine never reloads tables when Ln/Exp/Relu/Square interleave.
    * zero biases / unit gains (per the problem's fills) skip their ops at
      build time; general values still supported via the slow path.
  - Softmax max-subtraction is dropped (messages are bounded ~7, exp is safe;
    softmax is shift-invariant so the result matches the reference).
  - Final avg/max pooling is reduced per core on device; the cross-core
    combine and the last [1,64] @ [64,64] linear run on host.
"""

import math
import numpy as np
import ml_dtypes

import concourse.bass as bass
import concourse.bacc as bacc
import concourse.mybir as mybir
import concourse.tile as tile
from concourse.bass_utils import run_bass_kernel_spmd
from concourse.masks import make_identity

P = 128
NCORES = 8
N_NODES = 50000
N_EDGES = 1000000
HID = 64
L = 3
NBLK = 49                   # dst-node blocks per core
NPC = NBLK * P              # nodes per core (6272)
NTOT = NCORES * NPC         # padded node count (50176)
EPS_MSG = 1e-7
EPS_SM = 1e-16
LN_EPS = 1e-5

F32 = mybir.dt.float32
BF16 = mybir.dt.bfloat16
I16 = mybir.dt.int16
I32 = mybir.dt.int32
OP = mybir.AluOpType
ACT = mybir.ActivationFunctionType

# 6272 = 12*512 + 128
SLICES = [(i * 512, 512) for i in range(12)] + [(6144, 128)]


def _table_row(node):
    """Row of a node in the all-gathered z table (p-major per-core layout)."""
    kc = node // NPC
    loc = node - kc * NPC
    p = loc % P
    b = loc // P
    return kc * NPC + p * NBLK + b


def _prep_edges(edge_index, edge_attr):
    """Pair-packed table: 2 nodes per 256B row -> 25088 rows, unsigned int16
    idx covers the whole table. Per (core, block) edges are split by src-row
    parity (even rows first, then odd) so each chunk reads the low or high
    64 features of its gathered pair row via a pure AP offset."""
    src = edge_index[0].astype(np.int64)
    dst = edge_index[1].astype(np.int64)
    core = dst // NPC
    blk = (dst % NPC) // P
    dl = ((dst % NPC) % P).astype(np.float32)
    row = _table_row(src)
    pr = row >> 1               # pair row, 0..25087
    par = (row & 1).astype(np.int64)
    g = core * NBLK + blk
    av = edge_attr[:, 0].astype(np.float32)

    gh = g * 2 + par
    counts = np.bincount(gh, minlength=NCORES * NBLK * 2)
    cnt2 = counts.reshape(NCORES, NBLK, 2)
    cnt_e = cnt2[:, :, 0].max(axis=0).astype(int)
    cnt_o = cnt2[:, :, 1].max(axis=0).astype(int)
    K_E = [max(1, int(math.ceil(c / P))) for c in cnt_e]
    K_O = [max(1, int(math.ceil(c / P))) for c in cnt_o]
    choff = np.zeros(NBLK + 1, np.int64)
    for b in range(NBLK):
        choff[b + 1] = choff[b] + K_E[b] + K_O[b]
    nchk = int(choff[NBLK])
    # valid gather span per (block, parity half): trailing -1 slots in each
    # half-section are trimmed by num_idxs_reg
    reg = ([int(c) for c in cnt_e], [int(c) for c in cnt_o])

    order = np.argsort(gh, kind="stable")
    starts = np.zeros(NCORES * NBLK * 2 + 1, np.int64)
    np.cumsum(counts, out=starts[1:])
    pr_s, dl_s, av_s = pr[order], dl[order], av[order]

    nslot = nchk * P
    idx_c, dl_c, av_c = [], [], []
    for c in range(NCORES):
        idxa = np.full(nslot, -1, np.int32)
        dla = np.full(nslot, -1.0, np.float32)
        ava = np.zeros(nslot, np.float32)
        for b in range(NBLK):
            for half, K_half, cmax_a in ((0, K_E, cnt_e), (1, K_O, cnt_o)):
                gi = (c * NBLK + b) * 2 + half
                s, e = starts[gi], starts[gi + 1]
                n = e - s
                o = (choff[b] + (K_E[b] if half else 0)) * P
                cmax = cmax_a[b]
                idxa[o:o + cmax] = 0
                idxa[o:o + n] = pr_s[s:e]
                dla[o:o + n] = dl_s[s:e]
                ava[o:o + n] = av_s[s:e]
        # value arrays: slot j -> (lane j%128, chunk j//128)
        dl_c.append(np.ascontiguousarray(dla.reshape(nchk, P).T))
        av_c.append(np.ascontiguousarray(ava.reshape(nchk, P).T))
        # index array: slot j -> (partition j%16, free j//16), replicated x8
        i16 = idxa.astype(np.int16).reshape(-1, 16).T   # [16, nslot/16]
        idx_c.append(np.ascontiguousarray(np.tile(i16, (8, 1))))
    return K_E, K_O, reg, idx_c, dl_c, av_c


def _build_flags(data):
    return {
        "b_node": bool(np.any(data["b_node"])),
        "b_edge": bool(np.any(data["b_edge"])),
        "b1": bool(np.any(data["b1"])),
        "b2": bool(np.any(data["b2"])),
        "g1": bool(np.any(data["ln_g"] != 1.0) or np.any(data["ln_b"])),
        "gn": bool(np.any(data["norm_g"] != 1.0) or np.any(data["norm_b"])),
        "tval": float(data["t"][0]) if np.all(data["t"] == data["t"][0])
                else None,
    }


def _dma_gather128(g, out_ap, in_ap, idxs_ap, num_idxs, num_idxs_reg,
                   elem_size, elem_step, queue_num):
    """dma_gather for 128-byte elements with a 256-byte row stride.

    Replica of bass's dma_gather DRAM-source non-transpose path with the
    `elem_size_bytes % 256 == 0` assert dropped: that restriction only
    applies to the transpose (xbar) path — the non-transpose ucode handles
    any packet size, and the ISA stride field (stride_bytes_256) stays a
    256B multiple here (elem_step=128 bf16 elems = 256B)."""
    from concourse import ap_utils
    g._assert_queue_num(queue_num)
    assert idxs_ap.dtype == mybir.dt.int16
    assert in_ap.space == bass.MemorySpace.DRAM
    assert idxs_ap.space == bass.MemorySpace.SBUF
    assert out_ap.space == bass.MemorySpace.SBUF
    assert in_ap.dtype == out_ap.dtype
    assert ap_utils.ap_is_contiguous(out_ap.ap[1:])
    assert ap_utils.ap_is_contiguous(idxs_ap.ap[1:])
    assert in_ap.ap[-1][1] == out_ap.ap[-1][1] == elem_size
    assert out_ap.ap[0][1] * out_ap.ap[1][1] == \
        (num_idxs + P - 1) // P * P
    assert in_ap.ap[0][0] == elem_step
    stride_bytes = elem_step * mybir.dt.size(in_ap.dtype)
    assert stride_bytes % 256 == 0
    _in_ap = g.lower_ap_dma(in_ap, for_custom_bir_dma=True)
    _idxs_ap = g.lower_ap(idxs_ap)
    _out_ap = g.lower_ap(out_ap)
    return g.add_instruction(
        mybir.InstDMAGatherAnt(
            name=g.bass.get_next_instruction_name(),
            ins=[*_in_ap, _idxs_ap,
                 g.lower_val_access(g.to_reg(num_idxs_reg))],
            outs=[_out_ap],
            transpose=False,
            num_idxs=num_idxs,
            elem_size=elem_size,
            stride_bytes_256=stride_bytes // 256,
            gen_mode=0,
            single_packet=False,
            queue_num=queue_num,
            sbuf_tokens_per_rank=0,
            sbuf_free_dim_per_rank=0,
            sbuf_free_dim_pad_per_rank=0,
            sbuf_byte_offset=0,
        )
    )


def _pin_act_tables():
    """Make every ACT function resolve to natural_log_exp_and_others so the
    engine loads one table and never swaps (Ln/Exp/Relu/Square interleave
    heavily here). Only the set-choice hint is changed; ids stay aligned
    with act_info.json. Returns an undo callable."""
    orig = bacc.get_activation_tables
    PIN = "natural_log_exp_and_others"

    def patched(arch):
        tabs = orig(arch)
        if PIN not in tabs:
            return tabs
        keep = tabs[PIN]
        return {n: (f if n == PIN else (f - keep)) for n, f in tabs.items()}

    bacc.get_activation_tables = patched
    return lambda: setattr(bacc, "get_activation_tables", orig)


def _build(K_E, K_O, reg, flags):
    reg_e, reg_o = reg
    choff = [0]
    for b in range(NBLK):
        choff.append(choff[b] + K_E[b] + K_O[b])
    nchk = choff[NBLK]
    nslot = nchk * P
    KMAX = max(K_E[b] + K_O[b] for b in range(NBLK))
    HAS_BN = flags["b_node"]
    HAS_BE = flags["b_edge"]
    HAS_B1 = flags["b1"]
    HAS_B2 = flags["b2"]
    HAS_G1 = flags["g1"]
    HAS_GN = flags["gn"]
    TVAL = flags["tval"]

    unpin = _pin_act_tables()
    try:
        nc = bacc.Bacc("TRN2", target_bir_lowering=False, debug=False,
                       num_devices=NCORES, num_swdge_queues=4)
        d_idx = nc.dram_tensor("eidx16", [P, nslot // 16], I16,
                               kind="ExternalInput").ap()
        d_oh = nc.dram_tensor("ohm", [P, nchk, P], BF16,
                              kind="ExternalInput").ap()
        d_eaw = nc.dram_tensor("eaw", [P, nchk, HID], BF16,
                               kind="ExternalInput").ap()
        d_x16 = nc.dram_tensor("x16", [16, NPC], BF16,
                               kind="ExternalInput").ap()
        d_wnode = nc.dram_tensor("wnode", [16, HID], BF16,
                                 kind="ExternalInput").ap()
        d_w1 = nc.dram_tensor("w1", [L, HID, 2 * HID], BF16,
                              kind="ExternalInput").ap()
        d_w2 = nc.dram_tensor("w2", [L, 2 * HID, HID], BF16,
                              kind="ExternalInput").ap()
        if HAS_BN:
            d_bnode = nc.dram_tensor("bnode", [HID, 1], F32,
                                     kind="ExternalInput").ap()
        if HAS_BE:
            d_benm = nc.dram_tensor("benm", [P, HID], F32,
                                    kind="ExternalInput").ap()
        if HAS_B1:
            d_b1c = nc.dram_tensor("b1c", [2 * HID, L], F32,
                                   kind="ExternalInput").ap()
        if HAS_B2:
            d_b2c = nc.dram_tensor("b2c", [HID, L], F32,
                                   kind="ExternalInput").ap()
        if HAS_G1:
            d_g1c = nc.dram_tensor("g1c", [2 * HID, L], F32,
                                   kind="ExternalInput").ap()
            d_be1c = nc.dram_tensor("be1c", [2 * HID, L], F32,
                                    kind="ExternalInput").ap()
        if HAS_GN:
            d_ngc = nc.dram_tensor("ngc", [HID, L], F32,
                                   kind="ExternalInput").ap()
            d_nbc = nc.dram_tensor("nbc", [HID, L], F32,
                                   kind="ExternalInput").ap()
        if TVAL is None:
            d_tc = nc.dram_tensor("tcol", [P, L], F32,
                                  kind="ExternalInput").ap()
        d_out = nc.dram_tensor("pool_out", [HID, 2], F32,
                               kind="ExternalOutput").ap()

        with tile.TileContext(nc) as tc:
            with (
                tc.tile_pool(name="cpool", bufs=1) as cpool,
                tc.tile_pool(name="epool", bufs=3) as epool,
                tc.tile_pool(name="vpool", bufs=3) as vpool,
                tc.tile_pool(name="ohpool", bufs=3) as ohpool,
                tc.tile_pool(name="wpool", bufs=3) as wpool,
                tc.tile_pool(name="npool", bufs=3) as npool,
                tc.tile_pool(name="mpool", bufs=2) as mpool,
                tc.tile_pool(name="spool", bufs=2) as spool,
                tc.tile_pool(name="pmm", bufs=2, space="PSUM") as pmm,
                tc.tile_pool(name="pmlp", bufs=2, space="PSUM") as pmlp,
                tc.tile_pool(name="ptp", bufs=1, space="PSUM") as ptp,
                tc.tile_pool(name="paux", bufs=3, space="PSUM") as paux,
                tc.tile_pool(name="dpool", bufs=1, space="DRAM") as dpool,
            ):
                # ---- constants / persistent state ----
                ident = cpool.tile([P, P], F32)
                make_identity(nc, ident[:])
                identb = cpool.tile([P, P], BF16)
                nc.vector.tensor_copy(identb[:], ident[:])
                ones_row_bf = cpool.tile([1, P], BF16)
                nc.vector.memset(ones_row_bf[:], 1.0)
                inv64b = cpool.tile([P, 1], BF16)
                nc.vector.memset(inv64b[:], 1.0 / 64)
                inv128b = cpool.tile([P, 1], BF16)
                nc.vector.memset(inv128b[:], 1.0 / 128)
                epsm_col = cpool.tile([P, 1], F32)
                nc.vector.memset(epsm_col[:], EPS_MSG)
                lneps_col = cpool.tile([P, 1], F32)
                nc.vector.memset(lneps_col[:], LN_EPS)

                def load(shape, dtype, src_ap, name):
                    t = cpool.tile(shape, dtype, name=name)
                    nc.sync.dma_start(out=t[:], in_=src_ap)
                    return t

                idx16 = load([P, nslot // 16], I16, d_idx, "idx16_s")
                wnodeb = load([16, HID], BF16, d_wnode, "wnode_s")
                w1b = [load([HID, 2 * HID], BF16, d_w1[l], f"w1_{l}")
                       for l in range(L)]
                w2b = [load([2 * HID, HID], BF16, d_w2[l], f"w2_{l}")
                       for l in range(L)]
                bnode = load([HID, 1], F32, d_bnode, "bnode_s") if HAS_BN \
                    else None
                benm = load([P, HID], F32, d_benm, "benm_s") if HAS_BE \
                    else None
                b1c = load([2 * HID, L], F32, d_b1c, "b1c_s") if HAS_B1 \
                    else None
                b2c = load([HID, L], F32, d_b2c, "b2c_s") if HAS_B2 else None
                if HAS_G1:
                    g1c = load([2 * HID, L], F32, d_g1c, "g1c_s")
                    be1c = load([2 * HID, L], F32, d_be1c, "be1c_s")
                if HAS_GN:
                    ngc = load([HID, L], F32, d_ngc, "ngc_s")
                    nbc = load([HID, L], F32, d_nbc, "nbc_s")
                tcol = load([P, L], F32, d_tc, "tcol_s") if TVAL is None \
                    else None

                h = cpool.tile([HID, NPC], BF16)        # feature-major state
                zf = cpool.tile([HID, NPC], BF16)       # relu(LN(h)) scratch
                in1T_bf = cpool.tile([HID, NPC], BF16)  # MLP input, fmajor
                zrows = cpool.tile([P, NBLK * HID], F32)  # z node-major rows
                zbb = cpool.tile([P, NBLK, HID], BF16)  # z + b_edge (packed)
                nc.vector.memset(zbb[:], 0.0)
                gts = [cpool.tile([P, KMAX, HID], BF16, name=f"gt{i}")
                       for i in range(4)]
                for g_ in gts:
                    nc.vector.memset(g_[:], 0.0)

                zbounce = [dpool.tile([NPC, HID], BF16, name=f"zbounce{l}")
                           for l in range(L)]
                ztable = [dpool.tile([NTOT, HID], BF16, name=f"ztable{l}",
                                     addr_space="Shared") for l in range(L)]

                # ---- h0 = x @ W_node (+ b_node), feature-major bf16 ----
                x16b, x16_free = tc.tile([16, NPC], BF16, name="x16b")
                nc.sync.dma_start(out=x16b[:], in_=d_x16)
                for c0, w in SLICES:
                    ph0 = pmlp.tile([HID, 512], F32, name="p_h0", tag="mlp")
                    nc.tensor.matmul(ph0[:, :w], lhsT=wnodeb[:],
                                     rhs=x16b[:, c0:c0 + w],
                                     start=True, stop=True)
                    if HAS_BN:
                        nc.vector.tensor_scalar(
                            out=h[:, c0:c0 + w], in0=ph0[:, :w],
                            scalar1=bnode[:, 0:1], scalar2=None, op0=OP.add)
                    else:
                        nc.vector.tensor_copy(h[:, c0:c0 + w], ph0[:, :w])
                x16_free()

                # ---- feature-major LayerNorm + relu (stats over P dim) ----
                def ln_apply(dst, src, nfeat, g_col, b_col, c0, w, pfx):
                    invc = inv64b if nfeat == HID else inv128b
                    sq = spool.tile([nfeat, 512], BF16, name=pfx + "sq",
                                    tag=pfx + "sq")
                    nc.scalar.activation(sq[:, :w], src[:nfeat, c0:c0 + w],
                                         ACT.Square)
                    pm = paux.tile([1, 512], F32, name=pfx + "pm", tag="aux")
                    pq = paux.tile([1, 512], F32, name=pfx + "pq", tag="aux")
                    nc.tensor.matmul(pm[:, :w], lhsT=invc[:nfeat, :],
                                     rhs=src[:nfeat, c0:c0 + w],
                                     start=True, stop=True)   # mean
                    nc.tensor.matmul(pq[:, :w], lhsT=invc[:nfeat, :],
                                     rhs=sq[:, :w], start=True, stop=True)
                    tmp = spool.tile([1, 512], F32, name=pfx + "tmp",
                                     tag="sttmp")
                    nc.scalar.activation(tmp[:, :w], pm[:, :w], ACT.Square)
                    stA = spool.tile([1, 512], F32, name=pfx + "stA",
                                     tag="stA")
                    nc.vector.tensor_tensor(out=stA[:, :w], in0=pq[:, :w],
                                            in1=tmp[:, :w], op=OP.subtract)
                    # rstd = exp(-0.5*ln(var+eps)); both funcs in one table
                    nc.scalar.activation(stA[:, :w], stA[:, :w], ACT.Ln,
                                         bias=lneps_col[:1, :])
                    stAb = spool.tile([1, 512], BF16, name=pfx + "stAb",
                                      tag="stAb")
                    nc.scalar.activation(stAb[:, :w], stA[:, :w], ACT.Exp,
                                         scale=-0.5)
                    stBb = spool.tile([1, 512], BF16, name=pfx + "stBb",
                                      tag="stBb")
                    nc.vector.tensor_tensor(out=stBb[:, :w], in0=pm[:, :w],
                                            in1=stAb[:, :w], op=OP.mult)
                    pA = paux.tile([nfeat, 512], F32, name=pfx + "pA",
                                   tag="aux")
                    pB = paux.tile([nfeat, 512], F32, name=pfx + "pB",
                                   tag="aux")
                    nc.tensor.matmul(pA[:, :w], lhsT=ones_row_bf[:, :nfeat],
                                     rhs=stAb[:, :w], start=True, stop=True)
                    nc.tensor.matmul(pB[:, :w], lhsT=ones_row_bf[:, :nfeat],
                                     rhs=stBb[:, :w], start=True, stop=True)
                    u = spool.tile([nfeat, 512], BF16, name=pfx + "u",
                                   tag=pfx + "u")
                    nc.vector.tensor_tensor(out=u[:, :w],
                                            in0=src[:nfeat, c0:c0 + w],
                                            in1=pA[:, :w], op=OP.mult)
                    nc.vector.tensor_tensor(out=u[:, :w], in0=u[:, :w],
                                            in1=pB[:, :w], op=OP.subtract)
                    if g_col is None:
                        nc.scalar.activation(dst[:nfeat, c0:c0 + w],
                                             u[:, :w], ACT.Relu)
                    else:
                        nc.scalar.activation(dst[:nfeat, c0:c0 + w],
                                             u[:, :w], ACT.Relu,
                                             scale=g_col, bias=b_col)

                # ---- z-prep: transpose z to node-major rows + table pack
                def zprep_blocks(lay_z, b0, b1):
                    zsrc = h if lay_z == 0 else zf
                    for b in range(b0, b1):
                        ptz = ptp.tile([P, HID], BF16, name="ptz", tag="tp")
                        nc.tensor.transpose(ptz[:],
                                            zsrc[:, b * P:(b + 1) * P],
                                            identb[:HID, :HID])
                        nc.vector.tensor_copy(zrows[:, b * HID:(b + 1) * HID],
                                              ptz[:])
                    nb = b1 - b0
                    if HAS_BE:
                        nc.vector.tensor_tensor(
                            out=zbb[:, b0:b1, :],
                            in0=zrows[:, b0 * HID:b1 * HID]
                                .rearrange("p (b f) -> p b f", b=nb),
                            in1=benm[:].rearrange("p (o f) -> p o f", o=1)
                                .to_broadcast([P, nb, HID]),
                            op=OP.add)
                    else:
                        nc.scalar.activation(
                            zbb[:, b0:b1, :],
                            zrows[:, b0 * HID:b1 * HID]
                                .rearrange("p (b f) -> p b f", b=nb),
                            ACT.Copy)

                def table_publish(lay_z):
                    nc.sync.dma_start(
                        out=zbounce[lay_z][:]
                            .rearrange("(p b) f -> p (b f)", p=P),
                        in_=zbb[:])
                    nc.gpsimd.collective_compute(
                        "AllGather", OP.bypass,
                        replica_groups=[list(range(NCORES))],
                        ins=[zbounce[lay_z][:].opt()],
                        outs=[ztable[lay_z][:].opt()])

                # ---- layers ----
                for lay in range(L):
                    if lay == 0:
                        zprep_blocks(0, 0, NBLK)
                        table_publish(0)
                    tsc = (float(TVAL) if TVAL is not None
                           else tcol[:, lay:lay + 1])

                    # node MLP for a 512-node slice (interleaved into edges)
                    def mlp_slice(c0, w):
                        p1 = pmlp.tile([P, 512], F32, name="p1", tag="mlp")
                        nc.tensor.matmul(p1[:, :w], lhsT=w1b[lay][:],
                                         rhs=in1T_bf[:, c0:c0 + w],
                                         start=True, stop=True)
                        t1 = mpool.tile([P, 512], BF16, name="t1", tag="t1")
                        if HAS_B1:
                            nc.scalar.activation(t1[:, :w], p1[:, :w],
                                                 ACT.Identity,
                                                 bias=b1c[:, lay:lay + 1])
                        else:
                            nc.scalar.activation(t1[:, :w], p1[:, :w],
                                                 ACT.Copy)
                        r = mpool.tile([P, 512], BF16, name="mr", tag="mr")
                        ln_apply(r, t1, 2 * HID,
                                 g1c[:, lay:lay + 1] if HAS_G1 else None,
                                 be1c[:, lay:lay + 1] if HAS_G1 else None,
                                 0, w, "m")
                        p2 = pmlp.tile([HID, 512], F32, name="p2", tag="mlp")
                        nc.tensor.matmul(p2[:, :w], lhsT=w2b[lay][:],
                                         rhs=r[:, :w], start=True, stop=True)
                        if lay == 0:
                            if HAS_B2:
                                nc.vector.tensor_scalar(
                                    out=h[:, c0:c0 + w], in0=p2[:, :w],
                                    scalar1=b2c[:, 0:1], scalar2=None,
                                    op0=OP.add)
                            else:
                                nc.vector.tensor_copy(h[:, c0:c0 + w],
                                                      p2[:, :w])
                        else:
                            if HAS_B2:
                                conv = mpool.tile([HID, 512], BF16,
                                                  name="conv", tag="conv")
                                nc.vector.tensor_scalar(
                                    out=conv[:, :w], in0=p2[:, :w],
                                    scalar1=b2c[:, lay:lay + 1],
                                    scalar2=None, op0=OP.add)
                                nc.vector.tensor_tensor(
                                    out=h[:, c0:c0 + w],
                                    in0=h[:, c0:c0 + w],
                                    in1=conv[:, :w], op=OP.add)
                            else:
                                nc.vector.tensor_tensor(
                                    out=h[:, c0:c0 + w],
                                    in0=h[:, c0:c0 + w],
                                    in1=p2[:, :w], op=OP.add)

                    # ---- edge phase ----
                    for b in range(NBLK):
                        cbase = choff[b]
                        ke, ko = K_E[b], K_O[b]
                        Kb = ke + ko
                        gt = gts[b % 4]
                        tab = ztable[lay][:].rearrange(
                            "(m two) f -> m (two f)", two=2)
                        _dma_gather128(
                            nc.gpsimd, gt[:, 0:ke, :], tab[:, 0:HID],
                            idx16[:, cbase * 8:(cbase + ke) * 8],
                            num_idxs=ke * P, num_idxs_reg=int(reg_e[b]),
                            elem_size=HID, elem_step=2 * HID,
                            queue_num=(2 * b) % 4)
                        _dma_gather128(
                            nc.gpsimd, gt[:, ke:Kb, :], tab[:, HID:2 * HID],
                            idx16[:, (cbase + ke) * 8:(cbase + Kb) * 8],
                            num_idxs=ko * P, num_idxs_reg=int(reg_o[b]),
                            elem_size=HID, elem_step=2 * HID,
                            queue_num=(2 * b + 1) % 4)
                        eaw_t = wpool.tile([P, KMAX, HID], BF16, name="eawt",
                                           tag="eawt")
                        nc.sync.dma_start(
                            out=eaw_t[:, 0:Kb, :],
                            in_=d_eaw[:, cbase:cbase + Kb, :])
                        oh = ohpool.tile([P, KMAX, P], BF16, name="oh",
                                         tag="oh")
                        nc.sync.dma_start(
                            out=oh[:, 0:Kb, :],
                            in_=d_oh[:, cbase:cbase + Kb, :])
                        ea = epool.tile([P, KMAX, HID], BF16, name="ea",
                                        tag="ea")
                        val = vpool.tile([P, KMAX, 2, HID], BF16, name="val",
                                         tag="val")
                        eas = ea[:, 0:Kb, :]
                        nc.vector.tensor_tensor(
                            out=eas, in0=gt[:, 0:Kb, :],
                            in1=eaw_t[:, 0:Kb, :], op=OP.add)
                        nc.scalar.activation(eas, eas, ACT.Relu,
                                             bias=epsm_col[:])   # msg
                        nc.scalar.activation(val[:, 0:Kb, 0, :], eas,
                                             ACT.Exp, scale=tsc)
                        nc.vector.tensor_tensor(
                            out=val[:, 0:Kb, 1, :],
                            in0=val[:, 0:Kb, 0, :],
                            in1=eas, op=OP.mult)
                        ps = pmm.tile([P, P], F32, name="ps", tag="mm")
                        for k in range(Kb):
                            nc.tensor.matmul(ps[:], lhsT=oh[:, k, :],
                                             rhs=val[:, k, :, :],
                                             start=(k == 0),
                                             stop=(k == Kb - 1))
                        rec = npool.tile([P, HID], F32, name="rec", tag="rec")
                        nc.vector.tensor_scalar(out=rec[:], in0=ps[:, 0:HID],
                                                scalar1=EPS_SM, scalar2=None,
                                                op0=OP.add)
                        nc.vector.reciprocal(rec[:], rec[:])
                        in1 = npool.tile([P, HID], F32, name="in1", tag="in1")
                        nc.vector.tensor_tensor(out=in1[:],
                                                in0=ps[:, HID:2 * HID],
                                                in1=rec[:], op=OP.mult)
                        nc.vector.tensor_tensor(
                            out=in1[:], in0=in1[:],
                            in1=zrows[:, b * HID:(b + 1) * HID], op=OP.add)
                        pti = ptp.tile([HID, P], F32, name="pti", tag="tp")
                        nc.tensor.transpose(pti[:], in1[:], ident[:])
                        nc.scalar.activation(
                            in1T_bf[:, b * P:(b + 1) * P], pti[:], ACT.Copy)
                        # run the MLP for a 512-node slice as soon as its 4
                        # blocks of in1T are ready (overlaps later gathers),
                        # then LN + transpose those blocks for the next
                        # layer's z-table so only DMA+AllGather remain at
                        # the layer boundary
                        if b % 4 == 3 or b == NBLK - 1:
                            j = b // 4 if b % 4 == 3 else 12
                            mlp_slice(*SLICES[j])
                            if lay + 1 < L:
                                ln_apply(zf, h, HID,
                                         ngc[:, lay + 1:lay + 2]
                                         if HAS_GN else None,
                                         nbc[:, lay + 1:lay + 2]
                                         if HAS_GN else None,
                                         *SLICES[j], "z")
                                zprep_blocks(lay + 1, 4 * j,
                                             min(4 * j + 4, NBLK))
                    if lay + 1 < L:
                        table_publish(lay + 1)

                # ---- final norm + pooling ----
                for c0, w in SLICES:
                    ln_apply(zf, h, HID,
                             ngc[:, 0:1] if HAS_GN else None,
                             nbc[:, 0:1] if HAS_GN else None, c0, w, "z")
                poolsb = cpool.tile([HID, 2], F32)
                nc.vector.tensor_reduce(out=poolsb[:, 0:1], in_=zf[:HID, :],
                                        axis=mybir.AxisListType.X, op=OP.add)
                nc.vector.tensor_reduce(out=poolsb[:, 1:2], in_=zf[:HID, :],
                                        axis=mybir.AxisListType.X, op=OP.max)
                nc.sync.dma_start(out=d_out, in_=poolsb[:])

        nc.finalize()
    finally:
        unpin()
    return nc


def make_in_maps(data, idx_c, dl_c, av_c):
    flags = _build_flags(data)
    x = data["x"].astype(np.float32)
    xpad = np.zeros((NTOT, 16), np.float32)
    xpad[:N_NODES] = x
    w_edge = data["W_edge"].astype(np.float32)[0]          # [64]
    bf = ml_dtypes.bfloat16
    in_maps = []
    for c in range(NCORES):
        xc = xpad[c * NPC:(c + 1) * NPC]
        # feature-major x: [16, NPC] with node order (p-major per block)
        xp = np.ascontiguousarray(xc.T.astype(bf))   # [16, NPC], node = b*P+p
        eaw = np.ascontiguousarray(
            (av_c[c][:, :, None] * w_edge[None, None, :]).astype(bf))
        lanes = np.arange(P, dtype=np.float32)
        ohm = np.ascontiguousarray(
            (dl_c[c][:, :, None] == lanes[None, None, :]).astype(bf))
        m = {
            "eidx16": idx_c[c],
            "ohm": ohm,
            "eaw": eaw,
            "x16": xp,
            "wnode": np.ascontiguousarray(data["W_node"].astype(bf)),
            "w1": np.ascontiguousarray(data["W1"].astype(bf)),
            "w2": np.ascontiguousarray(data["W2"].astype(bf)),
        }
        if flags["b_node"]:
            m["bnode"] = data["b_node"].astype(np.float32)[:, None].copy()
        if flags["b_edge"]:
            m["benm"] = np.tile(data["b_edge"].astype(np.float32)[None, :],
                                (P, 1))
        if flags["b1"]:
            m["b1c"] = np.ascontiguousarray(data["b1"].astype(np.float32).T)
        if flags["b2"]:
            m["b2c"] = np.ascontiguousarray(data["b2"].astype(np.float32).T)
        if flags["g1"]:
            m["g1c"] = np.ascontiguousarray(data["ln_g"].astype(np.float32).T)
            m["be1c"] = np.ascontiguousarray(
                data["ln_b"].astype(np.float32).T)
        if flags["gn"]:
            m["ngc"] = np.ascontiguousarray(
                data["norm_g"].astype(np.float32).T)
            m["nbc"] = np.ascontiguousarray(
                data["norm_b"].astype(np.float32).T)
        if flags["tval"] is None:
            m["tcol"] = np.tile(data["t"].astype(np.float32)[None, :],
                                (P, 1))
        in_maps.append(m)
    return in_maps


def kernel(x, edge_attr, edge_index, W_node, b_node, W_edge, b_edge, t,
           W1, b1, ln_g, ln_b, W2, b2, norm_g, norm_b, W_lin, b_lin):
    data = dict(x=np.asarray(x), edge_attr=np.asarray(edge_attr),
                W_node=np.asarray(W_node), b_node=np.asarray(b_node),
                W_edge=np.asarray(W_edge), b_edge=np.asarray(b_edge),
                t=np.asarray(t), W1=np.asarray(W1), b1=np.asarray(b1),
                ln_g=np.asarray(ln_g), ln_b=np.asarray(ln_b),
                W2=np.asarray(W2), b2=np.asarray(b2),
                norm_g=np.asarray(norm_g), norm_b=np.asarray(norm_b))
    K_E, K_O, reg, idx_c, dl_c, av_c = _prep_edges(
        np.asarray(edge_index), np.asarray(edge_attr))
    nc = _build(K_E, K_O, reg, _build_flags(data))
    in_maps = make_in_maps(data, idx_c, dl_c, av_c)
    res = run_bass_kernel_spmd(nc, in_maps, core_ids=list(range(NCORES)))
    outs = np.stack([np.asarray(r["pool_out"], np.float32)
                     for r in res.results])        # [8, 64, 2]
    sums = outs[:, :, 0].sum(axis=0)
    maxs = outs[:, :, 1].max(axis=0)
    avg = (sums / float(N_NODES)).reshape(32, 2).mean(axis=1)
    mx = maxs.reshape(32, 2).max(axis=1)
    emb = np.concatenate([avg, mx])[None, :].astype(np.float32)
    out = emb @ np.asarray(W_lin, np.float32) + np.asarray(b_lin, np.float32)
    return out.astype(np.float32)
